# revision 2
# baseline (speedup 1.0000x reference)
"""Trainium2 Bass kernel: 4-layer dense transformer (B=2, T=2048, D=512, H=8, V=32000).

Sharding (DP2 x TP4 over 8 cores): core c handles batch b=c//4, TP rank r=c%4.
Per core: 2 attention heads (2r, 2r+1) over its whole batch, and an 8000-row
vocab shard of the final projection. Wo/LN/FFN run on all 2048 tokens of the
core's batch (replicated within the 4-core group) - this keeps the SPMD
program free of rank-dependent addressing; the only collective is one
AllGather of per-head attention outputs per layer within [[0-3],[4-7]].

Activations are kept transposed [D-partition, token-free]. Host-side prep in
kernel(): embedding gather + positional add, weight transposes, bf16 casts,
per-core slicing. Device work is bf16 matmuls with fp32 accumulation.
"""
import sys
sys.path.insert(0, "/opt/trn_rl_repo")
import numpy as np
import ml_dtypes

import concourse.bass as bass
import concourse.mybir as mybir
import concourse.tile as tile
from concourse import bacc
from concourse.bass_utils import run_bass_kernel_spmd
from concourse.masks import make_identity

F32 = mybir.dt.float32
BF16 = mybir.dt.bfloat16

N_CORES = 8
GROUPS = [[0, 1, 2, 3], [4, 5, 6, 7]]
P = 128
D = 512            # d_model
T = 2048           # tokens per batch (= per core)
H_PER = 2          # heads per core
DK = 64
L = 4              # layers
FF = 2048          # d_ff
VSH = 8000         # vocab shard per core
DC = D // P        # 4 D-chunks
TC = T // P        # 16 token chunks
TW = T // 512      # 4 token windows of 512
FC = FF // P       # 16 ff chunks
EPS = 1e-5
NEG = -1e30


def build_nc_full():
    nc = bacc.Bacc("TRN2", target_bir_lowering=False, debug=False,
                   num_devices=N_CORES)
    h0T = nc.declare_dram_parameter("h0T", [D, T], F32, isOutput=False)
    wqkvT = nc.declare_dram_parameter("wqkvT", [L, D, 3 * P], BF16, isOutput=False)
    woT = nc.declare_dram_parameter("woT", [L, D, D], BF16, isOutput=False)
    w1T = nc.declare_dram_parameter("w1T", [L, D, FF], BF16, isOutput=False)
    w2T = nc.declare_dram_parameter("w2T", [L, FF, D], BF16, isOutput=False)
    ln1g = nc.declare_dram_parameter("ln1g", [L, P, DC], F32, isOutput=False)
    ln1b = nc.declare_dram_parameter("ln1b", [L, P, DC], F32, isOutput=False)
    ln2g = nc.declare_dram_parameter("ln2g", [L, P, DC], F32, isOutput=False)
    ln2b = nc.declare_dram_parameter("ln2b", [L, P, DC], F32, isOutput=False)
    b1v = nc.declare_dram_parameter("b1v", [L, P, FC], F32, isOutput=False)
    b2v = nc.declare_dram_parameter("b2v", [L, P, DC], F32, isOutput=False)
    lnfg = nc.declare_dram_parameter("lnfg", [P, DC], F32, isOutput=False)
    lnfb = nc.declare_dram_parameter("lnfb", [P, DC], F32, isOutput=False)
    outwT = nc.declare_dram_parameter("outwT", [D, VSH], BF16, isOutput=False)
    logits = nc.declare_dram_parameter("logits", [T, VSH], F32, isOutput=True)

    from contextlib import ExitStack
    with tile.TileContext(nc) as tc:
        with ExitStack() as ctx:
            ep = ctx.enter_context
            const = ep(tc.tile_pool(name="const", bufs=1))
            hpool = ep(tc.tile_pool(name="hpool", bufs=1))
            awin = ep(tc.tile_pool(name="awin", bufs=1))
            qkvp = ep(tc.tile_pool(name="qkvp", bufs=1))
            wgt = ep(tc.tile_pool(name="wgt", bufs=1))
            wgt2 = ep(tc.tile_pool(name="wgt2", bufs=2))
            vecs = ep(tc.tile_pool(name="vecs", bufs=3))
            strow = ep(tc.tile_pool(name="strow", bufs=1))
            lnbig = ep(tc.tile_pool(name="lnbig", bufs=4))
            lnwin = ep(tc.tile_pool(name="lnwin", bufs=2))
            smallp = ep(tc.tile_pool(name="small", bufs=2))
            ptp = ep(tc.tile_pool(name="pt", bufs=17))
            vxp = ep(tc.tile_pool(name="vx", bufs=34))
            unp = ep(tc.tile_pool(name="un", bufs=3))
            utp = ep(tc.tile_pool(name="ut", bufs=1))
            utwp = ep(tc.tile_pool(name="utw", bufs=4))
            z1p = ep(tc.tile_pool(name="z1w", bufs=17))
            owp = ep(tc.tile_pool(name="ow", bufs=4))
            psm = ep(tc.tile_pool(name="ps", bufs=3, space="PSUM"))
            pstr = ep(tc.tile_pool(name="pstr", bufs=2, space="PSUM"))
            psov = ep(tc.tile_pool(name="psov", bufs=2, space="PSUM"))
            pstat = ep(tc.tile_pool(name="pst", bufs=1, space="PSUM"))
            dram = ep(tc.tile_pool(name="dram", bufs=2, space="DRAM"))
            # ---- constants ----
            ident = const.tile([P, P], BF16, tag="ident")
            make_identity(nc, ident)
            cmaskT = const.tile([P, P], F32, tag="cmaskT")
            nc.gpsimd.memset(cmaskT[:], 0.0)
            nc.gpsimd.affine_select(
                out=cmaskT[:], in_=cmaskT[:],
                compare_op=mybir.AluOpType.is_ge, fill=NEG,
                base=0, pattern=[[1, P]], channel_multiplier=-1,
            )
            mean_lhs = const.tile([P, 1], F32, tag="mean_lhs")
            nc.gpsimd.memset(mean_lhs[:], 1.0 / D)
            ones_row = const.tile([1, P], F32, tag="ones_row")
            nc.gpsimd.memset(ones_row[:], 1.0)
            eps_t = const.tile([P, 1], F32, tag="eps_t")
            nc.gpsimd.memset(eps_t[:], EPS)

            hT = [hpool.tile([P, T], F32, tag=f"hT{c}", name=f"hT{c}")
                  for c in range(DC)]
            for c in range(DC):
                nc.sync.dma_start(hT[c][:], h0T[c * P:(c + 1) * P, :])

            def load_vec(src, l, w, tag):
                t = vecs.tile([P, w], F32, tag=tag)
                nc.gpsimd.dma_start(t[:], src[l] if l is not None else src[:, :])
                return t

            def ln_window(g_t, b_t, w, out4, osl=None, pfx="ln"):
                """LayerNorm over D for token window w; writes 4 out tiles."""
                sl = slice(w * 512, (w + 1) * 512)
                s01 = lnbig.tile([P, 512], F32, tag="lnbig", name=f"{pfx}s01_{w}")
                s23 = lnbig.tile([P, 512], F32, tag="lnbig", name=f"{pfx}s23_{w}")
                nc.vector.tensor_add(s01[:], hT[0][:, sl], hT[1][:, sl])
                nc.vector.tensor_add(s23[:], hT[2][:, sl], hT[3][:, sl])
                nc.vector.tensor_add(s01[:], s01[:], s23[:])
                q0 = lnbig.tile([P, 512], F32, tag="lnbig", name=f"{pfx}q0_{w}")
                q1 = lnbig.tile([P, 512], F32, tag="lnbig", name=f"{pfx}q1_{w}")
                nc.vector.tensor_tensor(out=q0[:], in0=hT[0][:, sl],
                                        in1=hT[0][:, sl], op=mybir.AluOpType.mult)
                for c in range(1, DC):
                    nc.vector.tensor_tensor(out=q1[:], in0=hT[c][:, sl],
                                            in1=hT[c][:, sl],
                                            op=mybir.AluOpType.mult)
                    nc.vector.tensor_add(q0[:], q0[:], q1[:])
                mp = pstat.tile([1, 512], F32, space="PSUM", tag="st")
                nc.tensor.matmul(mp[:], mean_lhs[:], s01[:], start=True, stop=True)
                mu_row = strow.tile([1, 512], F32, tag="mu_row")
                nc.scalar.copy(mu_row[:], mp[:])
                mp2 = pstat.tile([1, 512], F32, space="PSUM", tag="st")
                nc.tensor.matmul(mp2[:], mean_lhs[:], q0[:], start=True, stop=True)
                ms_row = strow.tile([1, 512], F32, tag="ms_row")
                nc.scalar.copy(ms_row[:], mp2[:])
                bp = psm.tile([P, 512], F32, space="PSUM", tag="mm")
                nc.tensor.matmul(bp[:], ones_row[:], mu_row[:], start=True, stop=True)
                mu_bc = lnwin.tile([P, 512], F32, tag="mu_bc")
                nc.vector.tensor_copy(mu_bc[:], bp[:])
                bp2 = psm.tile([P, 512], F32, space="PSUM", tag="mm")
                nc.tensor.matmul(bp2[:], ones_row[:], ms_row[:], start=True, stop=True)
                rstd = lnwin.tile([P, 512], F32, tag="rstd")
                nc.vector.tensor_tensor(out=rstd[:], in0=mu_bc[:], in1=mu_bc[:],
                                        op=mybir.AluOpType.mult)
                nc.vector.tensor_tensor(out=rstd[:], in0=bp2[:], in1=rstd[:],
                                        op=mybir.AluOpType.subtract)
                nc.scalar.activation(rstd[:], rstd[:],
                                     mybir.ActivationFunctionType.Sqrt,
                                     bias=eps_t[:])
                nc.vector.reciprocal(rstd[:], rstd[:])
                for c in range(DC):
                    tt = smallp.tile([P, 512], F32, tag="ln_app")
                    nc.vector.tensor_tensor(out=tt[:], in0=hT[c][:, sl],
                                            in1=mu_bc[:],
                                            op=mybir.AluOpType.subtract)
                    nc.vector.tensor_tensor(out=tt[:], in0=tt[:], in1=rstd[:],
                                            op=mybir.AluOpType.mult)
                    dst = out4[c][:, osl] if osl is not None else out4[c][:]
                    nc.vector.tensor_scalar(
                        out=dst, in0=tt[:],
                        scalar1=g_t[:, c:c + 1], scalar2=b_t[:, c:c + 1],
                        op0=mybir.AluOpType.mult, op1=mybir.AluOpType.add)

            for l in range(L):
                g1 = load_vec(ln1g, l, DC, "g1")
                bb1 = load_vec(ln1b, l, DC, "bb1")
                g2 = load_vec(ln2g, l, DC, "g2")
                bb2 = load_vec(ln2b, l, DC, "bb2")
                fb1 = load_vec(b1v, l, FC, "fb1")
                fb2 = load_vec(b2v, l, DC, "fb2")
                wq_sb = [wgt2.tile([P, 3 * P], BF16, tag=f"wq{k}", name=f"wq{k}_{l}")
                         for k in range(DC)]
                wo_sb = [wgt2.tile([P, D], BF16, tag=f"wo{k}", name=f"wo{k}_{l}")
                         for k in range(DC)]
                w1_sb = [wgt.tile([P, FF], BF16, tag=f"w1{k}", name=f"w1{k}_{l}")
                         for k in range(DC)]
                w2_sb = [wgt.tile([P, D], BF16, tag=f"w2{k}", name=f"w2{k}_{l}")
                         for k in range(FC)]
                for k in range(DC):
                    nc.gpsimd.dma_start(wq_sb[k][:], wqkvT[l, k * P:(k + 1) * P, :])
                    nc.gpsimd.dma_start(wo_sb[k][:], woT[l, k * P:(k + 1) * P, :])
                    nc.gpsimd.dma_start(w1_sb[k][:], w1T[l, k * P:(k + 1) * P, :])
                for k in range(FC):
                    nc.gpsimd.dma_start(w2_sb[k][:], w2T[l, k * P:(k + 1) * P, :])

                # ---- LN1 + QKV, windowed ----
                qkv_sb = [qkvp.tile([P, T], BF16, tag=f"qkv{m}", name=f"qkv{m}_{l}")
                          for m in range(3)]
                for w in range(TW):
                    aw = [awin.tile([P, 512], BF16, tag=f"aw{c}",
                                    name=f"aw{c}_{l}_{w}") for c in range(DC)]
                    ln_window(g1, bb1, w, aw, pfx=f"l1_{l}")
                    for m in range(3):
                        pp = psm.tile([P, 512], F32, space="PSUM", tag="mm")
                        for k in range(DC):
                            nc.tensor.matmul(
                                pp[:], wq_sb[k][:, m * P:(m + 1) * P], aw[k][:],
                                start=(k == 0), stop=(k == DC - 1))
                        nc.scalar.copy(qkv_sb[m][:, w * 512:(w + 1) * 512], pp[:])

                # ---- attention, 2 heads ----
                uT = utp.tile([P, T], BF16, tag="uT")
                for h in range(H_PER):
                    hs = slice(h * DK, (h + 1) * DK)
                    vx = []
                    for kj in range(TC):
                        vt = pstr.tile([P, P], BF16, space="PSUM", tag="tr")
                        nc.tensor.transpose(
                            out=vt[:, :DK],
                            in_=qkv_sb[2][hs, kj * P:(kj + 1) * P],
                            identity=ident[hs, hs])
                        vxt = vxp.tile([P, DK + 1], BF16, tag="vx")
                        nc.vector.tensor_copy(vxt[:, :DK], vt[:, :DK])
                        nc.vector.memset(vxt[:, DK:DK + 1], 1.0)
                        vx.append(vxt)
                    for w in range(TW):
                        qsl = slice(w * 512, (w + 1) * 512)
                        pts = {}
                        for kj in range((w + 1) * 4):
                            sp = psm.tile([P, 512], F32, space="PSUM", tag="mm")
                            nc.tensor.matmul(
                                sp[:], qkv_sb[1][hs, kj * P:(kj + 1) * P],
                                qkv_sb[0][hs, qsl], start=True, stop=True)
                            if kj >= w * 4:
                                off = kj * P - w * 512
                                nc.vector.tensor_add(
                                    sp[:, off:off + P], sp[:, off:off + P],
                                    cmaskT[:])
                            pt = ptp.tile([P, 512], BF16, tag="pt")
                            nc.scalar.activation(
                                pt[:], sp[:], mybir.ActivationFunctionType.Exp,
                                bias=0.0, scale=0.125)
                            pts[kj] = pt
                        for qc in range(4):
                            qi = w * 4 + qc
                            op = psov.tile([P, DK + 1], F32, space="PSUM", tag="ov")
                            for kj in range(qi + 1):
                                nc.tensor.matmul(
                                    op[:], pts[kj][:, qc * P:(qc + 1) * P],
                                    vx[kj][:], start=(kj == 0), stop=(kj == qi))
                            rl = smallp.tile([P, 1], F32, tag="rl")
                            nc.vector.reciprocal(rl[:], op[:, DK:DK + 1])
                            un = unp.tile([P, P], BF16, tag="un")
                            nc.vector.tensor_scalar(
                                out=un[:, hs], in0=op[:, :DK],
                                scalar1=rl[:, :1], scalar2=None,
                                op0=mybir.AluOpType.mult)
                            tp = pstr.tile([P, P], BF16, space="PSUM", tag="tr")
                            nc.tensor.transpose(out=tp[:], in_=un[:],
                                                identity=ident[:])
                            nc.vector.tensor_copy(uT[hs, qi * P:(qi + 1) * P], tp[hs, :])

                # ---- AG#1 ----
                ag1_in = dram.tile([P, T], BF16, tag="ag1_in")
                ag1_out = dram.tile([4 * P, T], BF16, tag="ag1_out")
                nc.sync.dma_start(ag1_in[:], uT[:])
                nc.gpsimd.collective_compute(
                    "AllGather", mybir.AluOpType.bypass,
                    replica_groups=GROUPS,
                    ins=[ag1_in[:].opt()], outs=[ag1_out[:].opt()])

                # ---- Wo + residual; then LN2 + fused FFN; per window ----
                for n in range(TW):
                    sl = slice(n * 512, (n + 1) * 512)
                    utw = [utwp.tile([P, 512], BF16, tag="utw",
                                     name=f"utw{l}_{n}_{k2}") for k2 in range(DC)]
                    for k in range(DC):
                        nc.sync.dma_start(utw[k][:], ag1_out[k * P:(k + 1) * P, sl])
                    for m in range(DC):
                        pp = psm.tile([P, 512], F32, space="PSUM", tag="mm")
                        for k in range(DC):
                            nc.tensor.matmul(
                                pp[:], wo_sb[k][:, m * P:(m + 1) * P], utw[k][:],
                                start=(k == 0), stop=(k == DC - 1))
                        nc.vector.tensor_add(hT[m][:, sl], hT[m][:, sl], pp[:])
                for n in range(TW):
                    sl = slice(n * 512, (n + 1) * 512)
                    a2 = [awin.tile([P, 512], BF16, tag=f"a2w{c}",
                                    name=f"a2w{c}_{l}_{n}") for c in range(DC)]
                    ln_window(g2, bb2, n, a2, pfx=f"l2_{l}")
                    z1g = [z1p.tile([P, 512], BF16, tag="z1w",
                                     name=f"z1g{l}_{n}_{m}") for m in range(FC)]
                    for m in range(FC):
                        pp = psm.tile([P, 512], F32, space="PSUM", tag="mm")
                        for k in range(DC):
                            nc.tensor.matmul(
                                pp[:], w1_sb[k][:, m * P:(m + 1) * P], a2[k][:],
                                start=(k == 0), stop=(k == DC - 1))
                        nc.scalar.activation(
                            z1g[m][:], pp[:], mybir.ActivationFunctionType.Gelu,
                            bias=fb1[:, m:m + 1])
                    for md in range(DC):
                        pp = psm.tile([P, 512], F32, space="PSUM", tag="mm")
                        for k in range(FC):
                            nc.tensor.matmul(
                                pp[:], w2_sb[k][:, md * P:(md + 1) * P], z1g[k][:],
                                start=(k == 0), stop=(k == FC - 1))
                        tt = smallp.tile([P, 512], F32, tag="ffn_out")
                        nc.vector.tensor_scalar(
                            out=tt[:], in0=pp[:], scalar1=fb2[:, md:md + 1],
                            scalar2=None, op0=mybir.AluOpType.add)
                        nc.vector.tensor_add(hT[md][:, sl], hT[md][:, sl], tt[:])

            # ---- final LN + vocab-shard projection ----
            gf = load_vec(lnfg, None, DC, "gf")
            bf_t = load_vec(lnfb, None, DC, "bf")
            afT = [qkvp.tile([P, T], BF16, tag=f"qkv{c}", name=f"afT{c}")
                   for c in range(3)]
            afT.append(utp.tile([P, T], BF16, tag="uT", name="afT3"))
            for w in range(TW):
                ln_window(gf, bf_t, w, afT, osl=slice(w * 512, (w + 1) * 512),
                          pfx="lnf")
            NV = 500
            for vc in range(VSH // NV):
                ow_sb = [owp.tile([P, NV], BF16, tag="ow", name=f"ow{vc}_{k2}")
                         for k2 in range(DC)]
                for k in range(DC):
                    nc.gpsimd.dma_start(
                        ow_sb[k][:],
                        outwT[k * P:(k + 1) * P, vc * NV:(vc + 1) * NV])
                for tcx in range(TC):
                    pp = psm.tile([P, 512], F32, space="PSUM", tag="mm")
                    for k in range(DC):
                        nc.tensor.matmul(
                            pp[:, :NV], afT[k][:, tcx * P:(tcx + 1) * P],
                            ow_sb[k][:], start=(k == 0), stop=(k == DC - 1))
                    lo = smallp.tile([P, NV], F32, tag="lo", name=f"lo{vc}_{tcx}")
                    if tcx % 2 == 0:
                        nc.scalar.copy(lo[:], pp[:, :NV])
                    else:
                        nc.vector.tensor_copy(lo[:], pp[:, :NV])
                    nc.sync.dma_start(
                        logits[tcx * P:(tcx + 1) * P, vc * NV:(vc + 1) * NV],
                        lo[:])
    nc.compile()
    return nc



_NC_CACHE = None


def _get_nc():
    global _NC_CACHE
    if _NC_CACHE is None:
        _NC_CACHE = build_nc_full()
    return _NC_CACHE


def _vec_tile(v, chunks):
    # [chunks*128] -> [128, chunks] with [p, c] = v[c*128+p]
    return np.ascontiguousarray(np.asarray(v, np.float32).reshape(chunks, P).T)


def prepare_in_maps(inputs):
    return _prep(**inputs)


def _prep(x, embed_w, pos_w, ln1_g, ln1_b, Wqkv, Wo, ln2_g, ln2_b,
          W1, b1, W2, b2, lnf_g, lnf_b, out_w):
    x = np.asarray(x)
    embed_w = np.asarray(embed_w, np.float32)
    pos_w = np.asarray(pos_w, np.float32)
    Wqkv = np.asarray(Wqkv, np.float32)
    bf = ml_dtypes.bfloat16
    woT = np.ascontiguousarray(np.asarray(Wo).transpose(0, 2, 1)).astype(bf)
    w1T = np.ascontiguousarray(np.asarray(W1).transpose(0, 2, 1)).astype(bf)
    w2T = np.ascontiguousarray(np.asarray(W2).transpose(0, 2, 1)).astype(bf)
    ln_tiles = {
        "ln1g": np.stack([_vec_tile(np.asarray(ln1_g)[l], DC) for l in range(L)]),
        "ln1b": np.stack([_vec_tile(np.asarray(ln1_b)[l], DC) for l in range(L)]),
        "ln2g": np.stack([_vec_tile(np.asarray(ln2_g)[l], DC) for l in range(L)]),
        "ln2b": np.stack([_vec_tile(np.asarray(ln2_b)[l], DC) for l in range(L)]),
        "b1v": np.stack([_vec_tile(np.asarray(b1)[l], FC) for l in range(L)]),
        "b2v": np.stack([_vec_tile(np.asarray(b2)[l], DC) for l in range(L)]),
        "lnfg": _vec_tile(lnf_g, DC),
        "lnfb": _vec_tile(lnf_b, DC),
    }
    in_maps = []
    for c in range(N_CORES):
        b, r = c // 4, c % 4
        h0 = embed_w[x[b]] + pos_w[:T]                       # [T, D]
        h0T = np.ascontiguousarray(h0.T).astype(np.float32)  # [D, T]
        heads = [2 * r, 2 * r + 1]
        rows = np.concatenate([
            np.r_[heads[0] * DK:(heads[0] + 1) * DK,
                  heads[1] * DK:(heads[1] + 1) * DK] + w * D
            for w in range(3)])
        wqkvT = np.ascontiguousarray(
            Wqkv[:, rows, :].transpose(0, 2, 1)).astype(bf)  # [L, 512, 384]
        outwT = np.ascontiguousarray(
            np.asarray(out_w)[r * VSH:(r + 1) * VSH].T).astype(bf)
        m = {"h0T": h0T, "wqkvT": wqkvT, "woT": woT, "w1T": w1T, "w2T": w2T,
             "outwT": outwT}
        m.update(ln_tiles)
        in_maps.append(m)
    return in_maps


def assemble(results, inputs):
    out = np.empty((2, T, 4 * VSH), np.float32)
    for c in range(N_CORES):
        b, r = c // 4, c % 4
        out[b, :, r * VSH:(r + 1) * VSH] = results[c]["logits"]
    return out


def kernel(**inputs):
    nc = _get_nc()
    in_maps = prepare_in_maps(inputs)
    res = run_bass_kernel_spmd(nc, in_maps, list(range(N_CORES)))
    return assemble(res.results, inputs)



# revision 4
# speedup vs baseline: 1.0448x; 1.0448x over previous
"""Trainium2 Bass kernel v2: 4-layer dense transformer (B=2,T=2048,D=512,H=8,V=32000).

Sharding (DP2 x TP4 over 8 cores): core c handles batch b=c//4, TP rank r=c%4.
The residual stream h is TOKEN-SHARDED: each rank owns a 512-token window
[r*512,(r+1)*512). Per layer:
  LN1(own window) -> AllGather(aw, bf16)              [all tokens visible]
  QKV for the rank's 2 heads over all tokens; attention (causal, windowed)
  -> u [2 heads x 64, 2048] -> AllGather(u)
  -> rank extracts its own 512-token window via a register-driven dynamic
     column slice (offset from a per-core host input)  -> Wo + residual
  LN2(own window) -> FFN(own window) -> residual.
Final LN(own window) -> AllGather -> vocab-shard projection (8000/core).

LN gains/biases are folded into the adjacent weights/biases host-side.
Logits are written bf16 and upcast host-side (plus the lnf_b @ out_w row).
"""
import sys
sys.path.insert(0, "/opt/trn_rl_repo")
import numpy as np
import ml_dtypes

import concourse.bass as bass
import concourse.mybir as mybir
import concourse.tile as tile
from concourse import bacc
from concourse.bass_utils import run_bass_kernel_spmd
from concourse.masks import make_identity

F32 = mybir.dt.float32
F32R = mybir.dt.float32r
BF16 = mybir.dt.bfloat16
AF = mybir.ActivationFunctionType
OP = mybir.AluOpType

N_CORES = 8
GROUPS = [[0, 1, 2, 3], [4, 5, 6, 7]]
P = 128
D = 512            # d_model
T = 2048           # tokens per batch
WT = 512           # tokens per rank window
H_PER = 2          # heads per core
DK = 64
L = 4              # layers
FF = 2048          # d_ff
VSH = 8000         # vocab shard per core
DC = D // P        # 4 D-chunks
TC = T // P        # 16 token chunks
TW = T // WT       # 4 token windows
FC = FF // P       # 16 ff chunks
NV = 500           # vocab tile for the head
EPS = 1e-5
NEG = -1e30


def build_nc_full():
    nc = bacc.Bacc("TRN2", target_bir_lowering=False, debug=False,
                   num_devices=N_CORES)
    h0w = nc.declare_dram_parameter("h0w", [D, WT], F32, isOutput=False)
    wqkT = nc.declare_dram_parameter("wqkT", [L, D, 2 * P], BF16, isOutput=False)
    wvT = nc.declare_dram_parameter("wvT", [L, D, P], BF16, isOutput=False)
    woT = nc.declare_dram_parameter("woT", [L, D, D], BF16, isOutput=False)
    w1T = nc.declare_dram_parameter("w1T", [L, D, FF], BF16, isOutput=False)
    w2T = nc.declare_dram_parameter("w2T", [L, FF, D], BF16, isOutput=False)
    qkbv = nc.declare_dram_parameter("qkbv", [L, P, 2], F32, isOutput=False)
    vbr = nc.declare_dram_parameter("vbr", [L, 1, P], BF16, isOutput=False)
    fb1v = nc.declare_dram_parameter("fb1v", [L, P, FC], F32, isOutput=False)
    fb2v = nc.declare_dram_parameter("fb2v", [L, P, DC], F32, isOutput=False)
    outwT = nc.declare_dram_parameter("outwT", [D, VSH], BF16, isOutput=False)
    woff = nc.declare_dram_parameter("woff", [1, 1], mybir.dt.uint32,
                                     isOutput=False)
    logits = nc.declare_dram_parameter("logits", [T, VSH], BF16, isOutput=True)

    from contextlib import ExitStack
    with tile.TileContext(nc) as tc:
        with ExitStack() as ctx:
            ep = ctx.enter_context
            const = ep(tc.tile_pool(name="const", bufs=1))
            hpool = ep(tc.tile_pool(name="hpool", bufs=1))
            sqp = ep(tc.tile_pool(name="sqp", bufs=2))
            awp = ep(tc.tile_pool(name="awp", bufs=1))
            awfp = ep(tc.tile_pool(name="awfp", bufs=1))
            qkp = ep(tc.tile_pool(name="qkp", bufs=1))
            vxp = ep(tc.tile_pool(name="vxp", bufs=1))
            ptp = ep(tc.tile_pool(name="ptp", bufs=34))
            unp = ep(tc.tile_pool(name="unp", bufs=3))
            utp = ep(tc.tile_pool(name="utp", bufs=1))
            utwp = ep(tc.tile_pool(name="utwp", bufs=1))
            z1p = ep(tc.tile_pool(name="z1p", bufs=17))
            wgt = ep(tc.tile_pool(name="wgt", bufs=1))      # w1/w2
            wgt2 = ep(tc.tile_pool(name="wgt2", bufs=2))    # wqk/wv/wo
            vecs = ep(tc.tile_pool(name="vecs", bufs=2))
            strow = ep(tc.tile_pool(name="strow", bufs=1))
            smallp = ep(tc.tile_pool(name="small", bufs=2))
            owp = ep(tc.tile_pool(name="ow", bufs=2))
            lop = ep(tc.tile_pool(name="lo", bufs=4))
            psS = ep(tc.tile_pool(name="psS", bufs=2, space="PSUM"))
            psQ = ep(tc.tile_pool(name="psQ", bufs=2, space="PSUM"))
            psPV = ep(tc.tile_pool(name="psPV", bufs=2, space="PSUM"))
            pstat = ep(tc.tile_pool(name="pst", bufs=1, space="PSUM"))
            dram = ep(tc.tile_pool(name="dram", bufs=2, space="DRAM"))

            # ---- constants ----
            ident = const.tile([P, P], BF16, tag="ident")
            make_identity(nc, ident)
            # wide causal masks: cmw[j] is [P, WT] with NEG where the column
            # (q = w*WT + f) is behind the row's key token (k = (w*4+j)*P + p),
            # i.e. where f < j*P + p; zero elsewhere. Used as a start=True
            # matmul preload so score matmuls accumulate onto the mask.
            cmw = []
            for j in range(4):
                t = const.tile([P, WT], BF16, tag=f"cmw{j}", name=f"cmw{j}")
                nc.gpsimd.memset(t[:], 0.0)
                nc.gpsimd.affine_select(
                    out=t[:], in_=t[:],
                    compare_op=OP.is_ge, fill=NEG,
                    base=-j * P, pattern=[[1, WT]], channel_multiplier=-1,
                )
                cmw.append(t)
            mean_lhs = const.tile([P, 1], F32, tag="mean_lhs")
            nc.gpsimd.memset(mean_lhs[:], 1.0 / D)
            ones_f = const.tile([1, P], F32, tag="ones_f")
            nc.gpsimd.memset(ones_f[:], 1.0)
            ones_b = const.tile([1, P], BF16, tag="ones_b")
            nc.gpsimd.memset(ones_b[:], 1.0)
            eps_t = const.tile([1, 1], F32, tag="eps_t")
            nc.gpsimd.memset(eps_t[:], EPS)

            # own-window column offset (r*512) as a register for dyn slices
            eng = nc.sync
            woff_reg = eng.alloc_register("woff_reg")
            eng.reg_load(woff_reg, woff[0:1, 0:1])
            woff_v = eng.snap(woff_reg, min_val=0, max_val=T - WT)

            hw = [hpool.tile([P, WT], F32, tag=f"hw{c}", name=f"hw{c}")
                  for c in range(DC)]
            for c in range(DC):
                nc.sync.dma_start(hw[c][:], h0w[c * P:(c + 1) * P, :])

            def load_vec(src, l, w, tag):
                t = vecs.tile([P, w], F32, tag=tag, name=f"{tag}_{l}")
                nc.gpsimd.dma_start(t[:], src[l])
                return t

            def ln_own(out4, pfx, l):
                """LayerNorm (no gain/bias) of hw -> out4 (bf16 [P,WT] x4)."""
                mu_ps = pstat.tile([1, WT], F32, space="PSUM", tag="st",
                                   name=f"{pfx}mu_{l}")
                for c in range(DC):
                    nc.tensor.matmul(mu_ps[:], mean_lhs[:], hw[c][:],
                                     start=(c == 0), stop=(c == DC - 1))
                mu_sb = strow.tile([1, WT], F32, tag="mu_sb",
                                   name=f"{pfx}mu_sb_{l}")
                nc.vector.tensor_copy(mu_sb[:], mu_ps[:])
                ms_ps = pstat.tile([1, WT], F32, space="PSUM", tag="st",
                                   name=f"{pfx}ms_{l}")
                for c in range(DC):
                    sq = sqp.tile([P, WT], F32, tag="sq", name=f"{pfx}sq{c}_{l}")
                    nc.scalar.activation(sq[:], hw[c][:], AF.Square)
                    nc.tensor.matmul(ms_ps[:], mean_lhs[:], sq[:],
                                     start=(c == 0), stop=(c == DC - 1))
                var = strow.tile([1, WT], F32, tag="var", name=f"{pfx}var_{l}")
                nc.vector.tensor_tensor(out=var[:], in0=mu_sb[:], in1=mu_sb[:],
                                        op=OP.mult)
                nc.vector.tensor_tensor(out=var[:], in0=ms_ps[:], in1=var[:],
                                        op=OP.subtract)
                # rstd = exp(-0.5*ln(var+eps)) (stays in the exp/ln table set)
                nc.scalar.activation(var[:], var[:], AF.Ln, bias=eps_t[:])
                nc.scalar.activation(var[:], var[:], AF.Exp, scale=-0.5)
                mu_bc = psQ.tile([P, WT], F32, space="PSUM", tag="mm",
                                 name=f"{pfx}mub_{l}")
                nc.tensor.matmul(mu_bc[:], ones_f[:], mu_sb[:],
                                 start=True, stop=True)
                rs_bc = psQ.tile([P, WT], F32, space="PSUM", tag="mm",
                                 name=f"{pfx}rsb_{l}")
                nc.tensor.matmul(rs_bc[:], ones_f[:], var[:],
                                 start=True, stop=True)
                for c in range(DC):
                    tt = smallp.tile([P, WT], F32, tag="ln_tmp")
                    nc.vector.tensor_tensor(out=tt[:], in0=hw[c][:],
                                            in1=mu_bc[:], op=OP.subtract)
                    nc.vector.tensor_tensor(out=out4[c][:], in0=tt[:],
                                            in1=rs_bc[:], op=OP.mult)

            for l in range(L):
                qkb_t = load_vec(qkbv, l, 2, "qkb")
                fb1 = load_vec(fb1v, l, FC, "fb1")
                fb2 = load_vec(fb2v, l, DC, "fb2")
                vb_t = vecs.tile([1, P], BF16, tag="vb", name=f"vb_{l}")
                nc.gpsimd.dma_start(vb_t[:], vbr[l])
                wqk_sb = [wgt2.tile([P, 2 * P], BF16, tag=f"wqk{k}",
                                    name=f"wqk{k}_{l}") for k in range(DC)]
                wv_sb = [wgt2.tile([P, P], BF16, tag=f"wv{k}",
                                   name=f"wv{k}_{l}") for k in range(DC)]
                wo_sb = [wgt2.tile([P, D], BF16, tag=f"wo{k}",
                                   name=f"wo{k}_{l}") for k in range(DC)]
                w1_sb = [wgt.tile([P, FF], BF16, tag=f"w1{k}",
                                  name=f"w1{k}_{l}") for k in range(DC)]
                w2_sb = [wgt.tile([P, D], BF16, tag=f"w2{k}",
                                  name=f"w2{k}_{l}") for k in range(FC)]
                for k in range(DC):
                    nc.gpsimd.dma_start(wqk_sb[k][:], wqkT[l, k * P:(k + 1) * P, :])
                    nc.gpsimd.dma_start(wv_sb[k][:], wvT[l, k * P:(k + 1) * P, :])
                    nc.gpsimd.dma_start(wo_sb[k][:], woT[l, k * P:(k + 1) * P, :])
                    nc.gpsimd.dma_start(w1_sb[k][:], w1T[l, k * P:(k + 1) * P, :])
                for k in range(FC):
                    nc.gpsimd.dma_start(w2_sb[k][:], w2T[l, k * P:(k + 1) * P, :])

                # ---- LN1 (own window) -> aw; AllGather ----
                aw = [awp.tile([P, WT], BF16, tag=f"aw{c}", name=f"aw{c}_{l}")
                      for c in range(DC)]
                ln_own(aw, "l1", l)
                ag_in = dram.tile([D, WT], BF16, tag="ag_in")
                ag_out = dram.tile([TW * D, WT], BF16, tag="ag_out")
                for c in range(DC):
                    nc.sync.dma_start(ag_in[c * P:(c + 1) * P, :], aw[c][:])
                nc.gpsimd.collective_compute(
                    "AllGather", OP.bypass, replica_groups=GROUPS,
                    ins=[ag_in[:].opt()], outs=[ag_out[:].opt()])

                # ---- QKV over all tokens (rank's 2 heads) ----
                awf = [awfp.tile([P, T], BF16, tag=f"awf{c}", name=f"awf{c}_{l}")
                       for c in range(DC)]
                qk_sb = [qkp.tile([P, T], BF16, tag=f"qk{m}", name=f"qk{m}_{l}")
                         for m in range(2)]
                vx = [[vxp.tile([P, DK + 1], BF16, tag=f"vx{h}_{t}",
                                name=f"vx{h}_{t}_{l}") for t in range(TC)]
                      for h in range(H_PER)]
                if l == 0:
                    for h in range(H_PER):
                        for t in range(TC):
                            nc.vector.memset(vx[h][t][:, DK:DK + 1], 1.0)
                for w in range(TW):
                    wsl = slice(w * WT, (w + 1) * WT)
                    for c in range(DC):
                        nc.sync.dma_start(
                            awf[c][:, wsl],
                            ag_out[w * D + c * P:w * D + (c + 1) * P, :])
                    for m in range(2):
                        pp = psQ.tile([P, WT], F32, space="PSUM", tag="mm")
                        for k in range(DC):
                            nc.tensor.matmul(
                                pp[:], wqk_sb[k][:, m * P:(m + 1) * P],
                                awf[k][:, wsl], start=(k == 0),
                                stop=(k == DC - 1))
                        nc.vector.tensor_scalar(
                            out=qk_sb[m][:, wsl], in0=pp[:],
                            scalar1=qkb_t[:, m:m + 1], scalar2=None,
                            op0=OP.add)
                    for t in range(4):
                        ti = w * 4 + t
                        tsl = slice(ti * P, (ti + 1) * P)
                        vp = psPV.tile([P, P], F32, space="PSUM", tag="pv")
                        for k in range(DC):
                            nc.tensor.matmul(vp[:], awf[k][:, tsl], wv_sb[k][:],
                                             start=(k == 0), stop=False)
                        nc.tensor.matmul(vp[:], ones_b[:], vb_t[:],
                                         start=False, stop=True)
                        for h in range(H_PER):
                            nc.vector.tensor_copy(
                                vx[h][ti][:, :DK],
                                vp[:, h * DK:(h + 1) * DK])

                # ---- attention: scores+exp then PV, per window ----
                uT = utp.tile([P, T], BF16, tag="uT", name=f"uT_{l}")
                for w in range(TW):
                    qsl = slice(w * WT, (w + 1) * WT)
                    pts = {}
                    for h in range(H_PER):
                        hs = slice(h * DK, (h + 1) * DK)
                        for kj in range((w + 1) * 4):
                            sp = psS.tile([P, WT], F32, space="PSUM", tag="sc")
                            if kj >= w * 4:
                                j = kj - w * 4
                                nc.tensor.matmul(sp[:], ident[:], cmw[j][:],
                                                 start=True, stop=False)
                                nc.tensor.matmul(
                                    sp[:], qk_sb[1][hs, kj * P:(kj + 1) * P],
                                    qk_sb[0][hs, qsl], start=False, stop=True)
                            else:
                                nc.tensor.matmul(
                                    sp[:], qk_sb[1][hs, kj * P:(kj + 1) * P],
                                    qk_sb[0][hs, qsl], start=True, stop=True)
                            pt = ptp.tile([P, WT], BF16, tag="pt")
                            nc.scalar.activation(pt[:], sp[:], AF.Exp,
                                                 bias=0.0, scale=0.125)
                            pts[(h, kj)] = pt
                    for qc in range(4):
                        qi = w * 4 + qc
                        ops = []
                        for h in range(H_PER):
                            op = psPV.tile([P, DK + 1], F32, space="PSUM",
                                           tag="pv")
                            for kj in range(qi + 1):
                                nc.tensor.matmul(
                                    op[:], pts[(h, kj)][:, qc * P:(qc + 1) * P],
                                    vx[h][kj][:], start=(kj == 0),
                                    stop=(kj == qi))
                            ops.append(op)
                        un = unp.tile([P, P], BF16, tag="un")
                        for h in range(H_PER):
                            rl = smallp.tile([P, 1], F32, tag="rl")
                            nc.vector.reciprocal(rl[:], ops[h][:, DK:DK + 1])
                            nc.vector.tensor_scalar(
                                out=un[:, h * DK:(h + 1) * DK],
                                in0=ops[h][:, :DK], scalar1=rl[:, :1],
                                scalar2=None, op0=OP.mult)
                        tp = psS.tile([P, P], BF16, space="PSUM", tag="tr", bufs=1)
                        nc.tensor.transpose(out=tp[:], in_=un[:],
                                            identity=ident[:])
                        if qc % 2 == 0:
                            nc.vector.tensor_copy(uT[:, qi * P:(qi + 1) * P],
                                                  tp[:])
                        else:
                            nc.scalar.copy(uT[:, qi * P:(qi + 1) * P], tp[:])

                # ---- AllGather u; extract own window (dyn slice) ----
                agu_in = dram.tile([P, T], BF16, tag="agu_in")
                agu_out = dram.tile([TW * P, T], BF16, tag="agu_out")
                nc.sync.dma_start(agu_in[:], uT[:])
                nc.gpsimd.collective_compute(
                    "AllGather", OP.bypass, replica_groups=GROUPS,
                    ins=[agu_in[:].opt()], outs=[agu_out[:].opt()])
                utw = [utwp.tile([P, WT], BF16, tag=f"utw{k}",
                                 name=f"utw{k}_{l}") for k in range(DC)]
                for k in range(DC):
                    nc.sync.dma_start(
                        utw[k][:],
                        agu_out[k * P:(k + 1) * P, bass.ds(woff_v, WT)])

                # ---- Wo (own window) + residual ----
                for m in range(DC):
                    pp = psQ.tile([P, WT], F32, space="PSUM", tag="mm")
                    for k in range(DC):
                        nc.tensor.matmul(
                            pp[:], wo_sb[k][:, m * P:(m + 1) * P], utw[k][:],
                            start=(k == 0), stop=(k == DC - 1))
                    nc.vector.tensor_add(hw[m][:], hw[m][:], pp[:])

                # ---- LN2 + FFN (own window) ----
                a2 = [awp.tile([P, WT], BF16, tag=f"a2_{c}", name=f"a2{c}_{l}")
                      for c in range(DC)]
                ln_own(a2, "l2", l)
                z1 = [z1p.tile([P, WT], BF16, tag="z1", name=f"z1_{l}_{m}")
                      for m in range(FC)]
                for m in range(FC):
                    pp = psQ.tile([P, WT], F32, space="PSUM", tag="mm")
                    for k in range(DC):
                        nc.tensor.matmul(
                            pp[:], w1_sb[k][:, m * P:(m + 1) * P], a2[k][:],
                            start=(k == 0), stop=(k == DC - 1))
                    nc.scalar.activation(z1[m][:], pp[:], AF.Gelu,
                                         bias=fb1[:, m:m + 1])
                for md in range(DC):
                    pp = psQ.tile([P, WT], F32, space="PSUM", tag="mm")
                    for k in range(FC):
                        nc.tensor.matmul(
                            pp[:], w2_sb[k][:, md * P:(md + 1) * P], z1[k][:],
                            start=(k == 0), stop=(k == FC - 1))
                    tt = smallp.tile([P, WT], F32, tag="ffn_out")
                    nc.vector.tensor_scalar(
                        out=tt[:], in0=pp[:], scalar1=fb2[:, md:md + 1],
                        scalar2=None, op0=OP.add)
                    nc.vector.tensor_add(hw[md][:], hw[md][:], tt[:])

            # ---- final LN + AllGather + vocab-shard projection ----
            af = [awp.tile([P, WT], BF16, tag=f"a2_{c}", name=f"af{c}")
                  for c in range(DC)]
            ln_own(af, "lf", L)
            agf_in = dram.tile([D, WT], BF16, tag="ag_in")
            agf_out = dram.tile([TW * D, WT], BF16, tag="ag_out")
            for c in range(DC):
                nc.sync.dma_start(agf_in[c * P:(c + 1) * P, :], af[c][:])
            nc.gpsimd.collective_compute(
                "AllGather", OP.bypass, replica_groups=GROUPS,
                ins=[agf_in[:].opt()], outs=[agf_out[:].opt()])
            aft = [awfp.tile([P, T], BF16, tag=f"awf{c}", name=f"aft{c}")
                   for c in range(DC)]
            for w in range(TW):
                for c in range(DC):
                    nc.sync.dma_start(
                        aft[c][:, w * WT:(w + 1) * WT],
                        agf_out[w * D + c * P:w * D + (c + 1) * P, :])
            for vc in range(VSH // NV):
                ow_sb = [owp.tile([P, NV], BF16, tag=f"ow{k}",
                                  name=f"ow{vc}_{k}") for k in range(DC)]
                for k in range(DC):
                    nc.gpsimd.dma_start(
                        ow_sb[k][:],
                        outwT[k * P:(k + 1) * P, vc * NV:(vc + 1) * NV])
                for tcx in range(TC):
                    pp = psQ.tile([P, WT], F32, space="PSUM", tag="mm")
                    for k in range(DC):
                        nc.tensor.matmul(
                            pp[:, :NV], aft[k][:, tcx * P:(tcx + 1) * P],
                            ow_sb[k][:], start=(k == 0), stop=(k == DC - 1))
                    lo = lop.tile([P, NV], BF16, tag="lo",
                                  name=f"lo{vc}_{tcx}")
                    if tcx % 2 == 0:
                        nc.scalar.copy(lo[:], pp[:, :NV])
                    else:
                        nc.vector.tensor_copy(lo[:], pp[:, :NV])
                    nc.sync.dma_start(
                        logits[tcx * P:(tcx + 1) * P, vc * NV:(vc + 1) * NV],
                        lo[:])
    nc.compile()
    return nc


_NC_CACHE = None


def _get_nc():
    global _NC_CACHE
    if _NC_CACHE is None:
        _NC_CACHE = build_nc_full()
    return _NC_CACHE


def _vec_tile(v, chunks):
    # [chunks*128] -> [128, chunks] with [p, c] = v[c*128+p]
    return np.ascontiguousarray(np.asarray(v, np.float32).reshape(chunks, P).T)


def prepare_in_maps(inputs):
    return _prep(**inputs)


def _prep(x, embed_w, pos_w, ln1_g, ln1_b, Wqkv, Wo, ln2_g, ln2_b,
          W1, b1, W2, b2, lnf_g, lnf_b, out_w):
    bf = ml_dtypes.bfloat16
    x = np.asarray(x)
    embed_w = np.asarray(embed_w, np.float32)
    pos_w = np.asarray(pos_w, np.float32)
    Wqkv = np.asarray(Wqkv, np.float32)
    Wo_ = np.asarray(Wo, np.float32)
    W1_ = np.asarray(W1, np.float32)
    W2_ = np.asarray(W2, np.float32)
    ln1_g = np.asarray(ln1_g, np.float32)
    ln1_b = np.asarray(ln1_b, np.float32)
    ln2_g = np.asarray(ln2_g, np.float32)
    ln2_b = np.asarray(ln2_b, np.float32)
    b1_ = np.asarray(b1, np.float32)
    b2_ = np.asarray(b2, np.float32)
    lnf_g = np.asarray(lnf_g, np.float32)
    lnf_b = np.asarray(lnf_b, np.float32)
    out_w_ = np.asarray(out_w, np.float32)

    # fold LN2 gain into W1 (columns) and bias into b1
    w1g = W1_ * ln2_g[:, None, :]                    # [L, FF, D]
    b1f = b1_ + np.einsum("lfd,ld->lf", W1_, ln2_b)  # [L, FF]
    w1T = np.ascontiguousarray(w1g.transpose(0, 2, 1)).astype(bf)
    w2T = np.ascontiguousarray(W2_.transpose(0, 2, 1)).astype(bf)
    woT = np.ascontiguousarray(Wo_.transpose(0, 2, 1)).astype(bf)

    shared = {
        "fb1v": np.stack([_vec_tile(b1f[l], FC) for l in range(L)]),
        "fb2v": np.stack([_vec_tile(b2_[l], DC) for l in range(L)]),
        "w1T": w1T, "w2T": w2T, "woT": woT,
    }
    in_maps = []
    for c in range(N_CORES):
        b, r = c // 4, c % 4
        h0 = embed_w[x[b]] + pos_w[:T]                  # [T, D]
        h0w = np.ascontiguousarray(
            h0[r * WT:(r + 1) * WT].T).astype(np.float32)  # [D, WT]
        heads = [2 * r, 2 * r + 1]
        hrows = np.r_[heads[0] * DK:(heads[0] + 1) * DK,
                      heads[1] * DK:(heads[1] + 1) * DK]
        # fold LN1 gain into Wqkv cols, bias into per-out-dim bias
        wq = Wqkv[:, hrows, :] * ln1_g[:, None, :]            # [L,128,D] q rows
        wk = Wqkv[:, D + hrows, :] * ln1_g[:, None, :]
        wv = Wqkv[:, 2 * D + hrows, :] * ln1_g[:, None, :]
        bq = np.einsum("lhd,ld->lh", Wqkv[:, hrows, :], ln1_b)
        bk = np.einsum("lhd,ld->lh", Wqkv[:, D + hrows, :], ln1_b)
        bv = np.einsum("lhd,ld->lh", Wqkv[:, 2 * D + hrows, :], ln1_b)
        wqkT = np.ascontiguousarray(
            np.concatenate([wq, wk], axis=1).transpose(0, 2, 1)).astype(bf)
        wvp = np.zeros((L, D, 2 * (DK + 1)), np.float32)
        wvp[:, :, 0:DK] = wv.transpose(0, 2, 1)[:, :, 0:DK]
        wvp[:, :, DK + 1:2 * DK + 1] = wv.transpose(0, 2, 1)[:, :, DK:2 * DK]
        wvT_ = np.ascontiguousarray(wvp).astype(bf)
        qkbv = np.stack([np.stack([bq[l], bk[l]], axis=1) for l in range(L)])
        vbp = np.zeros((L, 1, 2 * (DK + 1)), np.float32)
        vbp[:, 0, 0:DK] = bv[:, 0:DK]
        vbp[:, 0, DK] = 1.0
        vbp[:, 0, DK + 1:2 * DK + 1] = bv[:, DK:2 * DK]
        vbp[:, 0, 2 * DK + 1] = 1.0
        vbr_ = np.ascontiguousarray(vbp).astype(bf)
        ow = out_w_[r * VSH:(r + 1) * VSH] * lnf_g[None, :]
        outwT_ = np.ascontiguousarray(ow.T).astype(bf)
        m = {"h0w": h0w, "wqkT": wqkT, "wvT": wvT_, "qkbv": qkbv,
             "vbr": vbr_, "outwT": outwT_,
             "woff": np.array([[r * WT]], np.uint32)}
        m.update(shared)
        in_maps.append(m)
    return in_maps


def assemble(results, inputs):
    lnf_b = np.asarray(inputs["lnf_b"], np.float32)
    out_w_ = np.asarray(inputs["out_w"], np.float32)
    out = np.empty((2, T, 4 * VSH), np.float32)
    for c in range(N_CORES):
        b, r = c // 4, c % 4
        row = out_w_[r * VSH:(r + 1) * VSH] @ lnf_b     # [VSH]
        out[b, :, r * VSH:(r + 1) * VSH] = (
            np.asarray(results[c]["logits"], np.float32) + row[None, :])
    return out


def kernel(**inputs):
    nc = _get_nc()
    in_maps = prepare_in_maps(inputs)
    res = run_bass_kernel_spmd(nc, in_maps, list(range(N_CORES)))
    return assemble(res.results, inputs)
